# revision 7
# baseline (speedup 1.0000x reference)
"""Local Gaussian refinement kernel for Trainium2 (8 NeuronCores, SPMD).

For each (b, k): round+clip the coarse coordinate, gather the 5x5 patch of
the heatmap around it, masked softmax over the 25 logits, return the
softmax-weighted expected (x, y).

Gather strategy: the HW SWDGE indirect-DMA unroll consumes exactly ONE
index per destination partition row and copies one contiguous run, so the
host pre-expands each heatmap slice into a sliding-window layout
  B5[s, y0, x, i] = heat[s, y0 + i, x]   (i in [0,5), rows padded to 196)
(a pure layout transform -- 5x data, built with numpy stride tricks).  In
B5 the whole 5x5 window at (y0, x0) is 25 CONTIGUOUS floats at element
offset (g*192 + y0)*1280 + x0*5, so one descriptor per (b,k) pair fetches
exactly the window: 3 indirect DMAs of [128, 25] (pairs are laid out
g = p + 128*t, t in {0,1,2}) move only 37.5 KB per core.

Latency discipline: the critical path is coords-DMA -> 6 DVE ops -> 3
SWDGE descgens -> tiny gathers -> per-chunk softmax -> triggered store.
Masks and expectation weights (window base folded in) are deferred into
the gather window via tile_wait_until; the softmax is processed per chunk
so only chunk 2's work remains after the last gather lands; the store is
a kv_writeback whose descriptors are prepared on the idle Pool engine
early (prepare_only) and fired by a trigger_dma once the result lands.

Sharding: data-parallel over batch; core m gets batches [16m, 16m+16).
Coords arrive pre-transposed as [128, 2*3]; pairs 272..383 are padding
(clamped + discarded).
"""

import sys

sys.path.insert(0, "/opt/trn_rl_repo")

import numpy as np

import concourse.bass as bass
import concourse.bacc as bacc
import concourse.tile as tile
from concourse import mybir
from concourse.bass_utils import run_bass_kernel_spmd

# Problem constants (hardcoded per contract).
B, K, H, W = 128, 17, 192, 256
NCORES = 8
BS = B // NCORES  # 16 batches per core
PAIRS = BS * K  # 272 (b,k) pairs per core
P = 128  # SBUF partitions
T = 3  # ceil(PAIRS / P) free-dim chunks
PADP = P * T  # 384 padded pairs
WN = 5  # window size (2*r+1)
SS = WN * WN  # 25 window elements
HP = H + 4  # padded rows in B5 (196)
SL = H * W * WN  # live elements per B5 slice (245760)
SLP = 1 << 18  # slice stride, padded to a power of two (disjoint-bit OR)
NEL5 = PAIRS * SLP  # B5 elements per core
BIGF = float(2 ** 23)  # RNE rounding trick constant
F32 = mybir.dt.float32
I32 = mybir.dt.int32
A = mybir.AluOpType
GOFF_MAX = (PAIRS - 1) * SLP  # clamp for padding pairs (g >= 272)


def build_program():
    # Bacc (not plain Bass): its compile() runs generate_event_semaphores,
    # which splits instructions with >1 semaphore wait (TRN2 HW limit).
    nc = bacc.Bacc(None, target_bir_lowering=False)
    heat = nc.dram_tensor("heat", [PAIRS * 256, 1024], F32, kind="ExternalInput")
    coords = nc.dram_tensor("coords", [P, T * 2], F32, kind="ExternalInput")
    out = nc.dram_tensor("out", [P, T * 2], F32, kind="ExternalOutput")

    with tile.TileContext(nc) as tc:
        with tc.tile_pool(name="sb", bufs=1) as pool:
            # ---- constants (iota on Pool, math on DVE) ----------------
            # goff[p, t] = min((p + 128t) * SLP, GOFF_MAX), exact in f32
            # (multiples of 2^18), converted to i32.  The within-slice
            # offset is < 2^18, so idx = goff BITWISE-OR rest is an exact
            # integer add (fp32 ALU adds would round above 2^24).
            g_i = pool.tile([P, T], I32)
            nc.gpsimd.iota(g_i[:], [[P, T]], base=0, channel_multiplier=1)
            gf = pool.tile([P, T], F32)
            nc.vector.tensor_copy(gf[:], g_i[:])
            nc.vector.tensor_scalar(gf[:], gf[:], float(SLP), float(GOFF_MAX), A.mult, A.min)
            goff = pool.tile([P, T], I32)
            nc.vector.tensor_copy(goff[:], gf[:])

            res = pool.tile([P, T * 2], F32)

            # B5 window data enumerates (j outer, i inner): position
            # k = 5*(x-x0) + i, so xoff varies on the OUTER window dim.
            xoff_i = pool.tile([P, T * SS], I32)  # value = j over (t,j,i)
            nc.gpsimd.iota(
                xoff_i[:], [[0, T], [1, WN], [0, WN]], base=0, channel_multiplier=0
            )
            xoff = pool.tile([P, T * SS], F32)
            nc.vector.tensor_copy(xoff[:], xoff_i[:])

            yoff_i = pool.tile([P, T * SS], I32)  # value = i over (t,j,i)
            nc.gpsimd.iota(
                yoff_i[:], [[0, T], [0, WN], [1, WN]], base=0, channel_multiplier=0
            )
            yoff = pool.tile([P, T * SS], F32)
            nc.vector.tensor_copy(yoff[:], yoff_i[:])

            # clip bounds per (t, c): (W-5, H-5) interleaved
            bnd_i = pool.tile([P, T * 2], I32)
            nc.gpsimd.iota(
                bnd_i[:],
                [[0, T], [-(W - H), 2]],
                base=W - WN,
                channel_multiplier=0,
            )
            bnd = pool.tile([P, T * 2], F32)
            nc.vector.tensor_copy(bnd[:], bnd_i[:])

            # ---- load coords (host pre-transposed to [128, 6]) --------
            crd = pool.tile([P, T * 2], F32)  # [p, (t,c)]
            nc.sync.dma_start(out=crd[:], in_=coords[:, :])

            # ---- critical index chain: 6 dependent DVE ops ------------
            # pxyb = coord + 2^23: biased round-to-nearest-even.  The
            # subtraction of (2^23 + 2) is exact, fusing round(c)-2 with
            # the high clip; max 0 completes base6 = (cbase, ry0)
            # interleaved.  Window offset in B5 is 5*(ry0*256 + cbase)
            # (exact in f32, < 2^18), converted to i32 and added to the
            # pre-clamped i32 slice offset goff.
            pxyb = pool.tile([P, T * 2], F32)
            nc.vector.tensor_scalar(pxyb[:], crd[:], BIGF, None, A.add)
            base6 = pool.tile([P, T * 2], F32)
            nc.vector.scalar_tensor_tensor(
                base6[:], pxyb[:], BIGF + 2.0, bnd[:], op0=A.subtract, op1=A.min
            )
            nc.vector.tensor_scalar(base6[:], base6[:], 0.0, None, A.max)
            bv = base6[:]
            cbase = bass.AP(bv.tensor, bv.offset, [bv.ap[0], [2, T]])  # x bases
            ry0 = bass.AP(bv.tensor, bv.offset + 1, [bv.ap[0], [2, T]])  # y bases

            idx0f = pool.tile([P, T], F32)  # ry0*W + cbase (window id)
            nc.vector.scalar_tensor_tensor(
                idx0f[:], ry0, float(W), cbase, op0=A.mult, op1=A.add
            )
            resti = pool.tile([P, T], I32)  # 5*(ry0*W + cbase)
            nc.vector.tensor_scalar(resti[:], idx0f[:], float(WN), None, A.mult)
            idx = pool.tile([P, T], I32)
            nc.vector.tensor_tensor(idx[:], resti[:], goff[:], op=A.bitwise_or)

            # ---- three gathers: one 25-elem descriptor per pair -------
            blk = pool.tile([P, T * SS], F32)
            for t in range(T):
                nc.gpsimd.indirect_dma_start(
                    out=blk[:, t * SS : (t + 1) * SS],
                    out_offset=None,
                    in_=heat[:, :],
                    in_offset=bass.IndirectOffsetOnAxis(
                        ap=idx[:, t : t + 1], axis=1
                    ),
                )

            # ---- masks + expectation weights, during the gathers ------
            # Deferred (tile_wait_until) so the scheduler cannot slot them
            # into the index chain above.
            with tc.tile_wait_until(0.004):
                # pdiff = base - round(coord) = (base6 + 2^23) - pxyb, exact
                pdiff = pool.tile([P, T * 2], F32)
                nc.vector.scalar_tensor_tensor(
                    pdiff[:], base6[:], BIGF, pxyb[:], op0=A.add, op1=A.subtract
                )
                pv = pdiff[:]
                ccp_b = bass.AP(pv.tensor, pv.offset, [pv.ap[0], [2, T], [0, SS]])
                rpy_b = bass.AP(pv.tensor, pv.offset + 1, [pv.ap[0], [2, T], [0, SS]])
                drow = pool.tile([P, T * SS], F32)
                nc.vector.tensor_tensor(
                    drow[:].rearrange("p (t s) -> p t s", s=SS),
                    rpy_b,
                    yoff[:].rearrange("p (t s) -> p t s", s=SS),
                    op=A.add,
                )
                dcol = pool.tile([P, T * SS], F32)
                nc.vector.tensor_tensor(
                    dcol[:].rearrange("p (t s) -> p t s", s=SS),
                    ccp_b,
                    xoff[:].rearrange("p (t s) -> p t s", s=SS),
                    op=A.add,
                )
                # valid <=> max(drow^2, dcol^2) <= 4.5 (integer-valued).
                # emsk is MULTIPLICATIVE (1 valid / 0 invalid): exp() runs
                # directly on the gathered logits and the mask zeroes the
                # moment inputs exactly.
                emsk = pool.tile([P, T * SS], F32)
                nc.vector.tensor_mul(drow[:], drow[:], drow[:])
                nc.vector.tensor_mul(dcol[:], dcol[:], dcol[:])
                nc.vector.tensor_tensor(emsk[:], drow[:], dcol[:], op=A.max)
                nc.vector.tensor_scalar(emsk[:], emsk[:], 4.5, None, A.is_le)

                # wxy[p, 50t+25c+s]: c=0 -> (cbase + j)*emsk,
                #                    c=1 -> (ry0 + i)*emsk
                cb_b = bass.AP(bv.tensor, bv.offset, [bv.ap[0], [2, T], [0, SS]])
                ry_b = bass.AP(bv.tensor, bv.offset + 1, [bv.ap[0], [2, T], [0, SS]])
                wxy = pool.tile([P, T * 2 * SS], F32)
                wv = wxy[:]
                wx_v = bass.AP(wv.tensor, wv.offset, [wv.ap[0], [2 * SS, T], [1, SS]])
                wy_v = bass.AP(
                    wv.tensor, wv.offset + SS, [wv.ap[0], [2 * SS, T], [1, SS]]
                )
                nc.vector.tensor_tensor(
                    wx_v, cb_b, xoff[:].rearrange("p (t s) -> p t s", s=SS), op=A.add
                )
                nc.vector.tensor_tensor(
                    wy_v, ry_b, yoff[:].rearrange("p (t s) -> p t s", s=SS), op=A.add
                )
                mv_ = emsk[:]
                emsk_b = bass.AP(
                    mv_.tensor, mv_.offset, [mv_.ap[0], [SS, T], [0, 2], [1, SS]]
                )
                nc.vector.tensor_tensor(
                    wxy[:].rearrange("p (t c s) -> p t c s", c=2, s=SS),
                    wxy[:].rearrange("p (t c s) -> p t c s", c=2, s=SS),
                    emsk_b,
                    op=A.mult,
                )

            # ---- per-chunk masked softmax moments ---------------------
            # Chunk t's work starts as soon as its gather lands, so only
            # chunk 2's exp/moments remain after the last DMA sem.  logits
            # bounded (|heat|<6) so exp() without the max-shift is safe.
            ez = pool.tile([P, T * SS], F32)
            mz = pool.tile([P, T * SS], F32)
            ssum = pool.tile([P, T], F32)
            q6 = pool.tile([P, T * 2 * SS], F32)
            num6 = pool.tile([P, T * 2], F32)
            for t in range(T):
                sl_s = slice(t * SS, (t + 1) * SS)
                nc.scalar.activation(
                    ez[:, sl_s], blk[:, sl_s], mybir.ActivationFunctionType.Exp
                )
                nc.vector.tensor_mul(mz[:, sl_s], ez[:, sl_s], emsk[:, sl_s])
                nc.vector.tensor_reduce(
                    ssum[:, t : t + 1],
                    mz[:, sl_s].rearrange("p (o s) -> p o s", s=SS),
                    axis=mybir.AxisListType.X,
                    op=A.add,
                )
                ev = ez[:, sl_s]
                ez_b = bass.AP(ev.tensor, ev.offset, [ev.ap[0], [0, 2], [1, SS]])
                sl_q = slice(t * 2 * SS, (t + 1) * 2 * SS)
                nc.vector.tensor_tensor(
                    q6[:, sl_q].rearrange("p (c s) -> p c s", s=SS),
                    ez_b,
                    wxy[:, sl_q].rearrange("p (c s) -> p c s", s=SS),
                    op=A.mult,
                )
                nc.vector.tensor_reduce(
                    num6[:, t * 2 : (t + 1) * 2],
                    q6[:, sl_q].rearrange("p (c s) -> p c s", s=SS),
                    axis=mybir.AxisListType.X,
                    op=A.add,
                )

            rinv = pool.tile([P, T], F32)
            nc.vector.reciprocal(rinv[:], ssum[:])
            rv = rinv[:]
            rinv_b = bass.AP(rv.tensor, rv.offset, [rv.ap[0], [1, T], [0, 2]])
            nc.vector.tensor_tensor(
                res[:].rearrange("p (t c) -> p t c", c=2),
                num6[:].rearrange("p (t c) -> p t c", c=2),
                rinv_b,
                op=A.mult,
            )

            # ---- store ------------------------------------------------
            nc.sync.dma_start(out=out[:, :], in_=res[:])
    nc.compile()
    return nc


_NC = None


def _get_nc():
    global _NC
    if _NC is None:
        _NC = build_program()
    return _NC


def make_in_maps(heatmaps: np.ndarray, coarse_coords: np.ndarray):
    heatmaps = np.ascontiguousarray(heatmaps, dtype=np.float32)
    coarse_coords = np.ascontiguousarray(coarse_coords, dtype=np.float32)
    in_maps = []
    for m in range(NCORES):
        hs = heatmaps[m * BS : (m + 1) * BS].reshape(PAIRS, H, W)
        # B5[g, y0, x, i] = hs[g, y0+i, x]: sliding rows (padded), window
        # (y0, x0) is then 25 contiguous floats at 5*(y0*W + x0).
        hp = np.pad(hs, ((0, 0), (0, WN - 1), (0, 0)))
        b5v = np.lib.stride_tricks.sliding_window_view(hp, WN, axis=1)
        b5 = np.zeros((PAIRS, SLP), dtype=np.float32)
        b5[:, : H * W * WN] = b5v.reshape(PAIRS, H * W * WN)
        b5 = b5.reshape(PAIRS * 256, 1024)
        cp = np.zeros((PADP, 2), dtype=np.float32)
        cp[:PAIRS] = coarse_coords[m * BS : (m + 1) * BS].reshape(PAIRS, 2)
        cs = np.ascontiguousarray(
            cp.reshape(T, P, 2).transpose(1, 0, 2).reshape(P, T * 2)
        )
        in_maps.append({"heat": b5, "coords": cs})
    return in_maps


def assemble_out(results) -> np.ndarray:
    outs = []
    for m in range(NCORES):
        r = results[m]["out"].reshape(P, T, 2).transpose(1, 0, 2).reshape(PADP, 2)
        outs.append(r[:PAIRS].reshape(BS, K, 2))
    return np.concatenate(outs, axis=0)


def kernel(heatmaps: np.ndarray, coarse_coords: np.ndarray) -> np.ndarray:
    nc = _get_nc()
    in_maps = make_in_maps(heatmaps, coarse_coords)
    results = run_bass_kernel_spmd(nc, in_maps, core_ids=list(range(NCORES)))
    return assemble_out(results.results)


# revision 8
# speedup vs baseline: 1.0197x; 1.0197x over previous
"""Local Gaussian refinement kernel for Trainium2 (8 NeuronCores, SPMD).

For each (b, k): round+clip the coarse coordinate, gather the 5x5 patch of
the heatmap around it, masked softmax over the 25 logits, return the
softmax-weighted expected (x, y).

Gather strategy: the HW SWDGE indirect-DMA unroll consumes exactly ONE
index per destination partition row and copies one contiguous run, so the
host pre-expands each heatmap slice into a sliding-window layout
  B5[s, y0, x, i] = heat[s, y0 + i, x]   (i in [0,5), rows padded to 196)
(a pure layout transform -- 5x data, built with numpy stride tricks).  In
B5 the whole 5x5 window at (y0, x0) is 25 CONTIGUOUS floats at element
offset (g*192 + y0)*1280 + x0*5, so one descriptor per (b,k) pair fetches
exactly the window: 3 indirect DMAs of [128, 25] (pairs are laid out
g = p + 128*t, t in {0,1,2}) move only 37.5 KB per core.

Latency discipline: the critical path is coords-DMA -> 6 DVE ops -> 3
SWDGE descgens -> tiny gathers -> per-chunk softmax -> triggered store.
Masks and expectation weights (window base folded in) are deferred into
the gather window via tile_wait_until; the softmax is processed per chunk
so only chunk 2's work remains after the last gather lands; the store is
a kv_writeback whose descriptors are prepared on the idle Pool engine
early (prepare_only) and fired by a trigger_dma once the result lands.

Sharding: data-parallel over batch; core m gets batches [16m, 16m+16).
Coords arrive pre-transposed as [128, 2*3]; pairs 272..383 are padding
(clamped + discarded).
"""

import sys

sys.path.insert(0, "/opt/trn_rl_repo")

import numpy as np

import concourse.bass as bass
import concourse.bacc as bacc
import concourse.tile as tile
from concourse import mybir
from concourse.bass_utils import run_bass_kernel_spmd

# Problem constants (hardcoded per contract).
B, K, H, W = 128, 17, 192, 256
NCORES = 8
BS = B // NCORES  # 16 batches per core
PAIRS = BS * K  # 272 (b,k) pairs per core
P = 128  # SBUF partitions
T = 3  # ceil(PAIRS / P) free-dim chunks
PADP = P * T  # 384 padded pairs
WN = 5  # window size (2*r+1)
SS = WN * WN  # 25 window elements
HP = H + 4  # padded rows in B5 (196)
SL = H * W * WN  # live elements per B5 slice (245760)
SLP = 1 << 18  # slice stride, padded to a power of two (disjoint-bit OR)
NEL5 = PAIRS * SLP  # B5 elements per core
BIGF = float(2 ** 23)  # RNE rounding trick constant
F32 = mybir.dt.float32
I32 = mybir.dt.int32
A = mybir.AluOpType
GOFF_MAX = (PAIRS - 1) * SLP  # clamp for padding pairs (g >= 272)


def build_program():
    # Bacc (not plain Bass): its compile() runs generate_event_semaphores,
    # which splits instructions with >1 semaphore wait (TRN2 HW limit).
    nc = bacc.Bacc(None, target_bir_lowering=False)
    # Drop the framework's unused const-tile initializers (bf16 1.0 and
    # uint8 127): nothing in this program reads them, and their two Pool
    # memsets serialize ahead of the startup barrier (~190 ns).
    blk0 = nc.m.functions[0].blocks[0]
    drop = [
        i
        for i in blk0.instructions
        if type(i).__name__ == "InstMemset"
        and any(
            t in str(i.outs[0]) for t in ("const-bfloat16-1.0", "const-uint8-127")
        )
    ]
    for i in drop:
        blk0.instructions.remove(i)
    heat = nc.dram_tensor("heat", [PAIRS * 256, 1024], F32, kind="ExternalInput")
    coords = nc.dram_tensor("coords", [P, T * 2], F32, kind="ExternalInput")
    out = nc.dram_tensor("out", [P, T * 2], F32, kind="ExternalOutput")

    with tile.TileContext(nc) as tc:
        with tc.tile_pool(name="sb", bufs=1) as pool:
            # ---- constants (iota on Pool, math on DVE) ----------------
            # goff[p, t] = min((p + 128t) * SLP, GOFF_MAX), exact in f32
            # (multiples of 2^18), converted to i32.  The within-slice
            # offset is < 2^18, so idx = goff BITWISE-OR rest is an exact
            # integer add (fp32 ALU adds would round above 2^24).
            g_i = pool.tile([P, T], I32)
            nc.gpsimd.iota(g_i[:], [[P, T]], base=0, channel_multiplier=1)
            gf = pool.tile([P, T], F32)
            nc.vector.tensor_copy(gf[:], g_i[:])
            nc.vector.tensor_scalar(gf[:], gf[:], float(SLP), float(GOFF_MAX), A.mult, A.min)
            goff = pool.tile([P, T], I32)
            nc.vector.tensor_copy(goff[:], gf[:])

            res = pool.tile([P, T * 2], F32)
            # ctx indices for the kv_writeback store: all zeros
            ctxz = pool.tile([P, 1], I32)
            nc.gpsimd.memset(ctxz[:], 0)

            # B5 window data enumerates (j outer, i inner): position
            # k = 5*(x-x0) + i, so xoff varies on the OUTER window dim.
            xoff_i = pool.tile([P, T * SS], I32)  # value = j over (t,j,i)
            nc.gpsimd.iota(
                xoff_i[:], [[0, T], [1, WN], [0, WN]], base=0, channel_multiplier=0
            )
            xoff = pool.tile([P, T * SS], F32)
            nc.vector.tensor_copy(xoff[:], xoff_i[:])

            yoff_i = pool.tile([P, T * SS], I32)  # value = i over (t,j,i)
            nc.gpsimd.iota(
                yoff_i[:], [[0, T], [0, WN], [1, WN]], base=0, channel_multiplier=0
            )
            yoff = pool.tile([P, T * SS], F32)
            nc.vector.tensor_copy(yoff[:], yoff_i[:])

            # clip bounds per (t, c): (W-5, H-5) interleaved
            bnd_i = pool.tile([P, T * 2], I32)
            nc.gpsimd.iota(
                bnd_i[:],
                [[0, T], [-(W - H), 2]],
                base=W - WN,
                channel_multiplier=0,
            )
            bnd = pool.tile([P, T * 2], F32)
            nc.vector.tensor_copy(bnd[:], bnd_i[:])

            # ---- load coords (host pre-transposed to [128, 6]) --------
            crd = pool.tile([P, T * 2], F32)  # [p, (t,c)]
            nc.sync.dma_start(out=crd[:], in_=coords[:, :])

            # ---- critical index chain: 6 dependent DVE ops ------------
            # pxyb = coord + 2^23: biased round-to-nearest-even.  The
            # subtraction of (2^23 + 2) is exact, fusing round(c)-2 with
            # the high clip; max 0 completes base6 = (cbase, ry0)
            # interleaved.  Window offset in B5 is 5*(ry0*256 + cbase)
            # (exact in f32, < 2^18), converted to i32 and added to the
            # pre-clamped i32 slice offset goff.
            pxyb = pool.tile([P, T * 2], F32)
            nc.vector.tensor_scalar(pxyb[:], crd[:], BIGF, None, A.add)
            base6 = pool.tile([P, T * 2], F32)
            nc.vector.scalar_tensor_tensor(
                base6[:], pxyb[:], BIGF + 2.0, bnd[:], op0=A.subtract, op1=A.min
            )
            nc.vector.tensor_scalar(base6[:], base6[:], 0.0, None, A.max)
            bv = base6[:]
            cbase = bass.AP(bv.tensor, bv.offset, [bv.ap[0], [2, T]])  # x bases
            ry0 = bass.AP(bv.tensor, bv.offset + 1, [bv.ap[0], [2, T]])  # y bases

            idx0f = pool.tile([P, T], F32)  # ry0*W + cbase (window id)
            nc.vector.scalar_tensor_tensor(
                idx0f[:], ry0, float(W), cbase, op0=A.mult, op1=A.add
            )
            resti = pool.tile([P, T], I32)  # 5*(ry0*W + cbase)
            nc.vector.tensor_scalar(resti[:], idx0f[:], float(WN), None, A.mult)
            idx = pool.tile([P, T], I32)
            nc.vector.tensor_tensor(idx[:], resti[:], goff[:], op=A.bitwise_or)

            # ---- three gathers: one 25-elem descriptor per pair -------
            blk = pool.tile([P, T * SS], F32)
            for t in range(T):
                nc.gpsimd.indirect_dma_start(
                    out=blk[:, t * SS : (t + 1) * SS],
                    out_offset=None,
                    in_=heat[:, :],
                    in_offset=bass.IndirectOffsetOnAxis(
                        ap=idx[:, t : t + 1], axis=1
                    ),
                )

            # ---- masks + expectation weights, during the gathers ------
            # Deferred (tile_wait_until) so the scheduler cannot slot them
            # into the index chain above.
            with tc.tile_wait_until(0.004):
                # pdiff = base - round(coord) = (base6 + 2^23) - pxyb, exact
                pdiff = pool.tile([P, T * 2], F32)
                nc.vector.scalar_tensor_tensor(
                    pdiff[:], base6[:], BIGF, pxyb[:], op0=A.add, op1=A.subtract
                )
                pv = pdiff[:]
                ccp_b = bass.AP(pv.tensor, pv.offset, [pv.ap[0], [2, T], [0, SS]])
                rpy_b = bass.AP(pv.tensor, pv.offset + 1, [pv.ap[0], [2, T], [0, SS]])
                drow = pool.tile([P, T * SS], F32)
                nc.vector.tensor_tensor(
                    drow[:].rearrange("p (t s) -> p t s", s=SS),
                    rpy_b,
                    yoff[:].rearrange("p (t s) -> p t s", s=SS),
                    op=A.add,
                )
                dcol = pool.tile([P, T * SS], F32)
                nc.vector.tensor_tensor(
                    dcol[:].rearrange("p (t s) -> p t s", s=SS),
                    ccp_b,
                    xoff[:].rearrange("p (t s) -> p t s", s=SS),
                    op=A.add,
                )
                # valid <=> max(drow^2, dcol^2) <= 4.5 (integer-valued).
                # emsk is MULTIPLICATIVE (1 valid / 0 invalid): exp() runs
                # directly on the gathered logits and the mask zeroes the
                # moment inputs exactly.
                emsk = pool.tile([P, T * SS], F32)
                nc.vector.tensor_mul(drow[:], drow[:], drow[:])
                nc.vector.tensor_mul(dcol[:], dcol[:], dcol[:])
                nc.vector.tensor_tensor(emsk[:], drow[:], dcol[:], op=A.max)
                nc.vector.tensor_scalar(emsk[:], emsk[:], 4.5, None, A.is_le)

                # wxy[p, 50t+25c+s]: c=0 -> (cbase + j)*emsk,
                #                    c=1 -> (ry0 + i)*emsk
                cb_b = bass.AP(bv.tensor, bv.offset, [bv.ap[0], [2, T], [0, SS]])
                ry_b = bass.AP(bv.tensor, bv.offset + 1, [bv.ap[0], [2, T], [0, SS]])
                wxy = pool.tile([P, T * 2 * SS], F32)
                wv = wxy[:]
                wx_v = bass.AP(wv.tensor, wv.offset, [wv.ap[0], [2 * SS, T], [1, SS]])
                wy_v = bass.AP(
                    wv.tensor, wv.offset + SS, [wv.ap[0], [2 * SS, T], [1, SS]]
                )
                nc.vector.tensor_tensor(
                    wx_v, cb_b, xoff[:].rearrange("p (t s) -> p t s", s=SS), op=A.add
                )
                nc.vector.tensor_tensor(
                    wy_v, ry_b, yoff[:].rearrange("p (t s) -> p t s", s=SS), op=A.add
                )
                mv_ = emsk[:]
                emsk_b = bass.AP(
                    mv_.tensor, mv_.offset, [mv_.ap[0], [SS, T], [0, 2], [1, SS]]
                )
                nc.vector.tensor_tensor(
                    wxy[:].rearrange("p (t c s) -> p t c s", c=2, s=SS),
                    wxy[:].rearrange("p (t c s) -> p t c s", c=2, s=SS),
                    emsk_b,
                    op=A.mult,
                )

            # ---- per-chunk masked softmax moments ---------------------
            # Chunk t's work starts as soon as its gather lands, so only
            # chunk 2's exp/moments remain after the last DMA sem.  logits
            # bounded (|heat|<6) so exp() without the max-shift is safe.
            ez = pool.tile([P, T * SS], F32)
            mz = pool.tile([P, T * SS], F32)
            ssum = pool.tile([P, T], F32)
            q6 = pool.tile([P, T * 2 * SS], F32)
            num6 = pool.tile([P, T * 2], F32)
            for t in range(T):
                sl_s = slice(t * SS, (t + 1) * SS)
                nc.scalar.activation(
                    ez[:, sl_s], blk[:, sl_s], mybir.ActivationFunctionType.Exp
                )
                nc.vector.tensor_mul(mz[:, sl_s], ez[:, sl_s], emsk[:, sl_s])
                nc.vector.tensor_reduce(
                    ssum[:, t : t + 1],
                    mz[:, sl_s].rearrange("p (o s) -> p o s", s=SS),
                    axis=mybir.AxisListType.X,
                    op=A.add,
                )
                ev = ez[:, sl_s]
                ez_b = bass.AP(ev.tensor, ev.offset, [ev.ap[0], [0, 2], [1, SS]])
                sl_q = slice(t * 2 * SS, (t + 1) * 2 * SS)
                nc.vector.tensor_tensor(
                    q6[:, sl_q].rearrange("p (c s) -> p c s", s=SS),
                    ez_b,
                    wxy[:, sl_q].rearrange("p (c s) -> p c s", s=SS),
                    op=A.mult,
                )
                nc.vector.tensor_reduce(
                    num6[:, t * 2 : (t + 1) * 2],
                    q6[:, sl_q].rearrange("p (c s) -> p c s", s=SS),
                    axis=mybir.AxisListType.X,
                    op=A.add,
                )

            rinv = pool.tile([P, T], F32)
            nc.vector.reciprocal(rinv[:], ssum[:])
            rv = rinv[:]
            rinv_b = bass.AP(rv.tensor, rv.offset, [rv.ap[0], [1, T], [0, 2]])
            nc.vector.tensor_tensor(
                res[:].rearrange("p (t c) -> p t c", c=2),
                num6[:].rearrange("p (t c) -> p t c", c=2),
                rinv_b,
                op=A.mult,
            )

            # ---- store: SWDGE prepare + trigger -----------------------
            # kv_writeback with batch=1, dhi=128, dho=1, ncn=n_ctx=6 and
            # ctx_idx=0 is an exact overwrite out[p, 0:6] = res[p, 0:6];
            # the trigger path skips the HWDGE DGE->DMA delay.
            ov = out[:, :]
            out4 = bass.AP(
                ov.tensor, 0, [[P * T * 2, 1], [T * 2, P], [T * 2, 1], [1, T * 2]]
            )
            rv2 = res[:]
            in4 = bass.AP(
                rv2.tensor, rv2.offset, [rv2.ap[0], [T * 2, 1], [T * 2, 1], [1, T * 2]]
            )
            store_sem = nc.alloc_semaphore("store_dma")
            prep = nc.gpsimd.kv_writeback(
                out4, in4, ctxz[:], prepare_only=True, sem=store_sem
            )
            # Drop the manual completion sem so Tile wires its own DMASW
            # tick into OnUpdate[0] (it skips doing so when the slot is
            # occupied, yet the epilogue still waits on that tick).
            prep.ins.sync_info.on_update.pop(0)
            nc.gpsimd.trigger_dma(count=None)
    nc.compile()
    return nc


_NC = None


def _get_nc():
    global _NC
    if _NC is None:
        _NC = build_program()
    return _NC


def make_in_maps(heatmaps: np.ndarray, coarse_coords: np.ndarray):
    heatmaps = np.ascontiguousarray(heatmaps, dtype=np.float32)
    coarse_coords = np.ascontiguousarray(coarse_coords, dtype=np.float32)
    in_maps = []
    for m in range(NCORES):
        hs = heatmaps[m * BS : (m + 1) * BS].reshape(PAIRS, H, W)
        # B5[g, y0, x, i] = hs[g, y0+i, x]: sliding rows (padded), window
        # (y0, x0) is then 25 contiguous floats at 5*(y0*W + x0).
        hp = np.pad(hs, ((0, 0), (0, WN - 1), (0, 0)))
        b5v = np.lib.stride_tricks.sliding_window_view(hp, WN, axis=1)
        b5 = np.zeros((PAIRS, SLP), dtype=np.float32)
        b5[:, : H * W * WN] = b5v.reshape(PAIRS, H * W * WN)
        b5 = b5.reshape(PAIRS * 256, 1024)
        cp = np.zeros((PADP, 2), dtype=np.float32)
        cp[:PAIRS] = coarse_coords[m * BS : (m + 1) * BS].reshape(PAIRS, 2)
        cs = np.ascontiguousarray(
            cp.reshape(T, P, 2).transpose(1, 0, 2).reshape(P, T * 2)
        )
        in_maps.append({"heat": b5, "coords": cs})
    return in_maps


def assemble_out(results) -> np.ndarray:
    outs = []
    for m in range(NCORES):
        r = results[m]["out"].reshape(P, T, 2).transpose(1, 0, 2).reshape(PADP, 2)
        outs.append(r[:PAIRS].reshape(BS, K, 2))
    return np.concatenate(outs, axis=0)


def kernel(heatmaps: np.ndarray, coarse_coords: np.ndarray) -> np.ndarray:
    nc = _get_nc()
    in_maps = make_in_maps(heatmaps, coarse_coords)
    results = run_bass_kernel_spmd(nc, in_maps, core_ids=list(range(NCORES)))
    return assemble_out(results.results)
